# revision 22
# baseline (speedup 1.0000x reference)
"""JK-GAMLP forward on 8 Trainium2 NeuronCores (Bass/Tile), v2.

Strategy: shard nodes across 8 cores; each core runs the whole per-node
network on node tiles of 512 (4 blocks of 128 on partitions, node =
128*?  no: node = p*4+b so each partition's 4 rows are contiguous in HBM
-> 2KB DMA descriptors).  4-stage software pipeline: P0 DMA loads run a
full round ahead of the P1 transposes so the PE never waits on HBM; P2
runs the JK MLP + attention scores with fp8 DoubleRow matmuls (weights
pre-scaled by 16 to stay in fp8-normal range, compensated exactly via
the ACT scale since prelu is positively homogeneous); P3 aggregates
hops with softmax weights (normalized up front by 1/sum via 4 tiny ops)
and runs the output FFN in bf16 with FWL weight loads.  Elementwise
work is split across ACT / DVE / GPSIMD to balance the three engines.
"""
import numpy as np

import concourse.bacc as bacc
import concourse.mybir as mybir
import concourse.tile as tile
from concourse.bass_utils import run_bass_kernel_spmd

AF = mybir.ActivationFunctionType
ALU = mybir.AluOpType
PM = mybir.MatmulPerfMode
AX = mybir.AxisListType
F32 = mybir.dt.float32
F32R = mybir.dt.float32r
FP8 = mybir.dt.float8e4
BF16 = mybir.dt.bfloat16

HOPS, F, HID, NCLS = 8, 128, 256, 64
N = 100000
NCORES = 8
NPC = 12544                       # nodes per core (padded: 8*12544 = 100352)
TILES = [(i * 512, 512) for i in range(24)] + [(12288, 256)]

WS = 16.0                         # fp8 weight pre-scale
WSI = 1.0 / WS

_CACHE = {}

# engine split knobs: per-hop engine for the xt PSUM->SBUF copies
# (GPSIMD cannot touch PSUM), and which blocks' aggregation chains run
# on GPSIMD as tensor_tensor mult/add pairs with 0-stride broadcast APs
# (GPSIMD rejects scalar-AP tensor_scalar ops).
XT_ENG = ["act", "act", "act", "act", "act", "dve", "dve", "dve"]
GP_AGG_BLOCKS = ()


def _build_program(loop_k=None):
    nc = bacc.Bacc("TRN2", target_bir_lowering=False, debug=False,
                   num_devices=NCORES)

    feats = nc.dram_tensor("feats", [HOPS, NPC, F], F32R, kind="ExternalInput")
    W_jk1 = nc.dram_tensor("W_jk1", [HOPS * F, HID], F32, kind="ExternalInput")
    W_jk2 = nc.dram_tensor("W_jk2", [HID, HID], F32, kind="ExternalInput")
    w_att_ref = nc.dram_tensor("w_att_ref", [HID], F32, kind="ExternalInput")
    w_att_x = nc.dram_tensor("w_att_x", [F], F32, kind="ExternalInput")
    W_o1 = nc.dram_tensor("W_o1", [F, HID], F32, kind="ExternalInput")
    W_o2 = nc.dram_tensor("W_o2", [HID, NCLS], F32, kind="ExternalInput")
    a_jk = nc.dram_tensor("a_jk", [1, 1], F32, kind="ExternalInput")
    a_main = nc.dram_tensor("a_main", [1, 1], F32, kind="ExternalInput")
    a_out = nc.dram_tensor("a_out", [1, 1], F32, kind="ExternalInput")
    out = nc.dram_tensor("out", [NPC, NCLS], F32, kind="ExternalOutput")

    ident = nc.inline_tensor(np.eye(128, dtype=np.float32), name="ident")

    with tile.TileContext(nc) as tc:
        with tc.tile_pool(name="const", bufs=1) as cpool, \
             tc.tile_pool(name="x", bufs=46) as xpool, \
             tc.tile_pool(name="xt", bufs=10) as xtpool, \
             tc.tile_pool(name="act", bufs=8) as actpool, \
             tc.tile_pool(name="sg", bufs=3) as sgpool, \
             tc.tile_pool(name="sm", bufs=10) as smpool, \
             tc.tile_pool(name="aggp", bufs=36) as aggpool, \
             tc.tile_pool(name="outp", bufs=3) as outpool, \
             tc.tile_pool(name="xt_ps", bufs=2, space="PSUM") as xtps, \
             tc.tile_pool(name="mm_ps", bufs=2, space="PSUM") as mmps, \
             tc.tile_pool(name="mm3_ps", bufs=2, space="PSUM") as mm3ps, \
             tc.tile_pool(name="e_ps", bufs=2, space="PSUM") as eps:

            # ---------------- setup: weights + constants ----------------
            id_sb = cpool.tile([128, 128], F32)
            nc.sync.dma_start(id_sb[:], ident[:])

            # W_jk1 -> fp8 pairs [p, hop_pair, ko, c] scaled by 16
            # (stage f32 chunks through the x pool to save persistent SBUF)
            w1p = cpool.tile([128, 4, 2, HID], FP8)
            for hp in range(4):
                stg = xpool.tile([128, 2, HID], F32, tag="x")
                nc.sync.dma_start(
                    stg[:], W_jk1.ap().rearrange("(hp ko p) c -> p hp ko c",
                                                 hp=4, ko=2)[:, hp])
                nc.vector.tensor_scalar(w1p[:, hp], stg[:], WS, None,
                                        op0=ALU.mult)

            # W_jk2 -> fp8 [p, ko, c] scaled by 16
            w2f = xpool.tile([128, 2, HID], F32, tag="x")
            nc.sync.dma_start(
                w2f[:], W_jk2.ap().rearrange("(ko p) c -> p ko c", ko=2))
            w2p = cpool.tile([128, 2, HID], FP8)
            nc.vector.tensor_scalar(w2p[:], w2f[:], WS, None, op0=ALU.mult)

            # w_att_ref -> fp8 [p, ko, 8] (replicated to 8 cols), scaled
            wreff = cpool.tile([128, 2], F32)
            nc.sync.dma_start(wreff[:],
                              w_att_ref.ap().rearrange("(ko p) -> p ko", ko=2))
            # padded to [.., 2, 16] so the DoubleRow LDW ko-stride is 16B
            wrefp_t = cpool.tile([128, 2, 16], FP8)
            for j in range(8):
                nc.vector.tensor_scalar(wrefp_t[:, :, j], wreff[:], WS, None,
                                        op0=ALU.mult)
            wrefp = wrefp_t[:, :, 0:8]

            # w_att_x -> fp8 [p, hp, ko, 8] one-hot per hop, scaled
            watxf = cpool.tile([128, 1], F32)
            nc.sync.dma_start(watxf[:],
                              w_att_x.ap().rearrange("(p o) -> p o", o=1))
            watxp = cpool.tile([128, 4, 2, 16], FP8)
            nc.vector.memset(watxp[:], 0.0)
            for h in range(HOPS):
                hp, ko = divmod(h, 2)
                nc.vector.tensor_scalar(watxp[:, hp, ko, h:h + 1], watxf[:],
                                        WS, None, op0=ALU.mult)

            # W_o1 f32r [p, m, c]
            wo1f = xpool.tile([128, 2, 128], F32, tag="x")
            nc.sync.dma_start(
                wo1f[:], W_o1.ap().rearrange("p (m c) -> p m c", m=2))
            wo1r = cpool.tile([128, 2, 128], F32R)
            nc.vector.tensor_copy(wo1r[:], wo1f[:])

            # W_o2 bf16 [p, k, c]
            wo2f = xpool.tile([128, 2, NCLS], F32, tag="x")
            nc.sync.dma_start(
                wo2f[:], W_o2.ap().rearrange("(k p) c -> p k c", p=128))
            wo2b = cpool.tile([128, 2, NCLS], BF16)
            nc.vector.tensor_copy(wo2b[:], wo2f[:])

            # replicate the three PReLU alphas to [128, 3] via K=1 matmul
            al_f = cpool.tile([1, 3], F32)
            nc.sync.dma_start(al_f[0:1, 0:1], a_jk[:])
            nc.sync.dma_start(al_f[0:1, 1:2], a_main[:])
            nc.sync.dma_start(al_f[0:1, 2:3], a_out[:])
            ones_sb = cpool.tile([1, 128], F32)
            nc.vector.memset(ones_sb[:], 1.0)
            half_col = cpool.tile([128, 1], F32)
            nc.vector.memset(half_col[:], 0.5)
            al_ps = mmps.tile([128, 3], F32, tag="mm")
            nc.tensor.matmul(al_ps[:], ones_sb[:], al_f[:],
                             start=True, stop=True)
            alpha = cpool.tile([128, 3], F32)
            nc.scalar.activation(alpha[:], al_ps[:], AF.Copy)

            # ---------------- 4-stage pipeline ----------------
            # P0: DMA loads; P1: transposes + fp8 copies; P2: MLP + scores
            # + softmax weights; P3: aggregation + output FFN + store.

            def phase0(st):
                n0, TT = st["n0"], st["TT"]
                B = TT // 128
                x_sb = []
                for h in range(HOPS):
                    x_h = xpool.tile([128, B, 128], F32R, tag="x")
                    nc.sync.dma_start(
                        x_h[:],
                        feats.ap()[h, n0:n0 + TT, :].rearrange(
                            "(p b) f -> p b f", b=B))
                    x_sb.append(x_h)
                st["x"] = x_sb

            def phase1(st):
                TT = st["TT"]
                B = TT // 128
                x_sb = st["x"]
                xtp = []
                for hp in range(4):
                    pair = xtpool.tile([128, 2, TT], FP8, tag="xt")
                    for ko in range(2):
                        h = hp * 2 + ko
                        ps = xtps.tile([128, TT], F32R, tag="xtps")
                        for b in range(B):
                            nc.tensor.transpose(ps[:, b * 128:(b + 1) * 128],
                                                x_sb[h][:, b, :],
                                                id_sb[:].bitcast(F32R))
                        if XT_ENG[h] == "act":
                            nc.scalar.activation(pair[:, ko, :], ps[:],
                                                 AF.Copy)
                        else:
                            nc.vector.tensor_copy(pair[:, ko, :], ps[:])
                    xtp.append(pair)
                st["xt"] = xtp

            def phase2(st):
                TT = st["TT"]
                B = TT // 128
                xtp = st["xt"]

                # h1 = prelu(concat @ (16*W1)) / 16, fp8 DoubleRow over hops
                h1p = actpool.tile([128, 2, TT], FP8, tag="h1")
                for m in range(2):
                    ps = mmps.tile([128, TT], F32, tag="mm")
                    for hp in range(4):
                        nc.tensor.matmul(ps[:],
                                         w1p[:, hp, :, m * 128:(m + 1) * 128],
                                         xtp[hp][:, :, :],
                                         start=(hp == 0), stop=(hp == 3),
                                         perf_mode=PM.DoubleRow)
                    nc.scalar.activation(h1p[:, m, :], ps[:], AF.Prelu,
                                         scale=WSI, alpha=alpha[:, 0:1])

                # jk = prelu(h1 @ (16*W2)) / 16, one DoubleRow matmul per half
                jkp = actpool.tile([128, 2, TT], FP8, tag="jk")
                for m in range(2):
                    ps = mmps.tile([128, TT], F32, tag="mm")
                    nc.tensor.matmul(ps[:], w2p[:, :, m * 128:(m + 1) * 128],
                                     h1p[:, :, :], start=True, stop=True,
                                     perf_mode=PM.DoubleRow)
                    nc.scalar.activation(jkp[:, m, :], ps[:], AF.Prelu,
                                         scale=WSI, alpha=alpha[:, 1:2])

                # scores: s_ps = 16*(jk@wref + x@watx)  [8, TT]
                s_ps = mmps.tile([8, TT], F32, tag="mm")
                nc.tensor.matmul(s_ps[:], wrefp, jkp[:, :, :],
                                 start=True, stop=False,
                                 perf_mode=PM.DoubleRow,
                                 skip_group_check=True)
                for hp in range(4):
                    nc.tensor.matmul(s_ps[:], watxp[:, hp, :, 0:8],
                                     xtp[hp][:, :, :],
                                     start=False, stop=(hp == 3),
                                     perf_mode=PM.DoubleRow,
                                     skip_group_check=True)

                # sigmoid(s) = 0.5 + 0.5*tanh(s/2); then softmax over hops.
                sg = sgpool.tile([8, TT], F32, tag="sg")
                nc.scalar.activation(sg[:], s_ps[:], AF.Tanh, scale=0.5 * WSI)
                e_ps = eps.tile([128, B * 8], F32, tag="eps")
                for b in range(B):
                    nc.tensor.transpose(e_ps[:, b * 8:(b + 1) * 8],
                                        sg[:, b * 128:(b + 1) * 128],
                                        id_sb[0:8, 0:8])
                e_sb = smpool.tile([128, B, 8], F32, tag="e")
                nc.scalar.activation(e_sb[:], e_ps[:], AF.Exp,
                                     scale=0.5, bias=half_col[:])
                esum = smpool.tile([128, B], F32, tag="esum")
                nc.vector.tensor_reduce(esum[:], e_sb[:], AX.X, ALU.add)
                r_sb = smpool.tile([128, B], F32, tag="r")
                nc.vector.reciprocal(r_sb[:], esum[:])
                ew = smpool.tile([128, B, 8], F32, tag="ew")
                for b in range(B):
                    nc.vector.tensor_scalar(ew[:, b, :], e_sb[:, b, :],
                                            r_sb[:, b:b + 1], None,
                                            op0=ALU.mult)
                st["ew"] = ew

            def phase3(st):
                n0, TT = st["n0"], st["TT"]
                B = TT // 128
                x_sb, ew = st["x"], st["ew"]

                agg_blocks = []
                for b in range(B):
                    if b in GP_AGG_BLOCKS:
                        # GPSIMD chain: mult with 0-stride broadcast, then add
                        cur = None
                        for h in range(HOPS):
                            tmp = aggpool.tile([128, 128], F32, tag="aggp")
                            nc.gpsimd.tensor_tensor(
                                tmp[:], x_sb[h][:, b, :].bitcast(F32),
                                ew[:, b, h:h + 1].broadcast_to([128, 128]),
                                op=ALU.mult)
                            if cur is None:
                                cur = tmp
                            else:
                                nxt = aggpool.tile([128, 128],
                                                   F32R if h == HOPS - 1
                                                   else F32, tag="aggp")
                                nc.gpsimd.tensor_add(nxt[:], cur[:], tmp[:])
                                cur = nxt
                        agg_blocks.append(cur)
                        continue
                    cur = aggpool.tile([128, 128], F32, tag="aggp")
                    nc.vector.tensor_scalar(cur[:],
                                            x_sb[0][:, b, :].bitcast(F32),
                                            ew[:, b, 0:1], None, op0=ALU.mult)
                    for h in range(1, HOPS):
                        nxt = aggpool.tile([128, 128],
                                           F32R if h == HOPS - 1 else F32,
                                           tag="aggp")
                        nc.vector.scalar_tensor_tensor(
                            nxt[:], x_sb[h][:, b, :].bitcast(F32),
                            ew[:, b, h:h + 1], cur[:],
                            op0=ALU.mult, op1=ALU.add)
                        cur = nxt
                    agg_blocks.append(cur)

                a_ps = mm3ps.tile([128, TT], F32R, tag="mm3")
                for b in range(B):
                    nc.tensor.transpose(a_ps[:, b * 128:(b + 1) * 128],
                                        agg_blocks[b][:],
                                        id_sb[:].bitcast(F32R))
                aggt = actpool.tile([128, TT], F32R, tag="aggt")
                nc.scalar.activation(aggt[:], a_ps[:], AF.Copy)

                # o1 = prelu(agg @ W_o1) in bf16
                o1p = actpool.tile([128, 2, TT], BF16, tag="o1")
                for m in range(2):
                    ps = mm3ps.tile([128, TT], F32, tag="mm3")
                    nc.tensor.matmul(ps[:], wo1r[:, m, :], aggt[:],
                                     start=True, stop=True)
                    nc.scalar.activation(o1p[:, m, :], ps[:], AF.Prelu,
                                         alpha=alpha[:, 2:3])

                o_ps = mm3ps.tile([128, B * NCLS], F32, tag="mm3")
                for b in range(B):
                    for k in range(2):
                        nc.tensor.matmul(
                            o_ps[:, b * NCLS:(b + 1) * NCLS],
                            o1p[:, k, b * 128:(b + 1) * 128],
                            wo2b[:, k, :],
                            start=(k == 0), stop=(k == 1),
                            skip_group_check=True)
                out_sb = outpool.tile([128, B, NCLS], F32, tag="out")
                nc.scalar.activation(out_sb[:], o_ps[:], AF.Copy)
                nc.sync.dma_start(
                    out.ap()[n0:n0 + TT, :].rearrange("(p b) c -> p b c",
                                                      b=B),
                    out_sb[:])

            import contextlib
            loop_cm = tc.For_i(0, loop_k) if loop_k else contextlib.nullcontext()
            NT = len(TILES)
            states = {}
            with loop_cm:
                for t in range(NT + 3):
                    if t < NT:
                        n0, TT = TILES[t]
                        states[t] = {"n0": n0, "TT": TT}
                    # P1 first: consumes DMAs issued a full round earlier
                    if t >= 1 and t - 1 < NT:
                        phase1(states[t - 1])
                    if t >= 2 and t - 2 < NT:
                        phase2(states[t - 2])
                    if t >= 3 and t - 3 < NT:
                        phase3(states[t - 3])
                        del states[t - 3]
                    if t < NT:
                        phase0(states[t])

    nc.compile()
    return nc


def _get_program():
    if "nc" not in _CACHE:
        _CACHE["nc"] = _build_program()
    return _CACHE["nc"]


def kernel(**inputs):
    nc = _get_program()

    feats = np.asarray(inputs["feats"], dtype=np.float32)
    pad = NCORES * NPC - feats.shape[1]
    feats_p = np.pad(feats, ((0, 0), (0, pad), (0, 0)))

    def scal(name):
        return np.asarray(inputs[name], dtype=np.float32).reshape(1, 1)

    shared = {
        "W_jk1": np.ascontiguousarray(inputs["W_jk1"], dtype=np.float32),
        "W_jk2": np.ascontiguousarray(inputs["W_jk2"], dtype=np.float32),
        "w_att_ref": np.ascontiguousarray(inputs["w_att_ref"], dtype=np.float32),
        "w_att_x": np.ascontiguousarray(inputs["w_att_x"], dtype=np.float32),
        "W_o1": np.ascontiguousarray(inputs["W_o1"], dtype=np.float32),
        "W_o2": np.ascontiguousarray(inputs["W_o2"], dtype=np.float32),
        "a_jk": scal("a_jk"), "a_main": scal("a_main"), "a_out": scal("a_out"),
    }
    in_maps = []
    for c in range(NCORES):
        m = dict(shared)
        m["feats"] = np.ascontiguousarray(feats_p[:, c * NPC:(c + 1) * NPC, :])
        in_maps.append(m)

    res = run_bass_kernel_spmd(nc, in_maps, core_ids=list(range(NCORES)))
    out = np.concatenate([res.results[c]["out"] for c in range(NCORES)],
                         axis=0)[:N]
    return np.ascontiguousarray(out, dtype=np.float32)
